# revision 31
# baseline (speedup 1.0000x reference)
"""AttentionBlock (GroupNorm -> 1x1 QKV -> softmax attention -> proj -> residual)
on Trainium2, data-parallel over batch: 32 images across 8 NeuronCores (4 per core).

Self-contained: hardcodes B=32, C=256, H=W=32, GROUPS=8, EPS=1e-5.

Algebra: with M = Wq^T Wk folded on the host, logits l[n,m] = h[:,n]^T M h[:,m]
need only one intermediate t = M h instead of both q and k.  Attention only
mixes spatially, so proj folds into V: Wp (attn (Wv h)) = attn ((Wp Wv) h).
The output is produced directly in [c, n] layout (outT = vto^T @ PT), so no
PE transposes are needed; the softmax denominator is broadcast to all
partitions by an all-ones fp8 stationary matmul and divided out with
reciprocal + tensor-tensor multiply.

fp8 (e4m3) with MatmulPerfMode.DoubleRow (contracts 2 k-tiles per
instruction at 0.5 cycles/row) runs the t/logits/AV/den matmuls at 4x bf16
throughput.  V rides fp8 + an fp8 residual correction (vto ~ vto8 + vtoe8,
two accumulating DR passes) to keep the output error ~1e-3-level.
"""

import numpy as np
import ml_dtypes
import jax
from jax.experimental.shard_map import shard_map
from jax.sharding import Mesh, PartitionSpec

import concourse.bass as bass
import concourse.tile as tile
from concourse import bacc, mybir
from concourse import bass2jax

F32 = mybir.dt.float32
BF16 = mybir.dt.bfloat16
FP8 = mybir.dt.float8e4
AF = mybir.ActivationFunctionType
ALU = mybir.AluOpType
DR = mybir.MatmulPerfMode.DoubleRow

NCORES = 8
B = 32
BPC = B // NCORES  # images per core
C = 256
N = 1024           # H*W
G = 8              # groups
GS = C // G        # 32 channels per group
EPS = 1e-5
P = 128
NT = C // P        # 2 channel tiles
WSCALE = 16.0      # host prescale on M and WpWv (fp8 range/precision)
SCALE = (C ** -0.5) / WSCALE   # exp scale absorbs the M prescale
EBIAS = -3.0       # exp bias: keeps exp() inside fp8 range; cancels in softmax

_cached = None


def _build_program(repeat=1):
    nc = bacc.Bacc("TRN2", target_bir_lowering=False, debug=False,
                   num_devices=NCORES)

    x_d = nc.dram_tensor("x", [BPC, C, N], F32, kind="ExternalInput")
    mt8_d = nc.dram_tensor("mt8", [P, NT, C], FP8, kind="ExternalInput")
    wvT_d = nc.dram_tensor("wvT", [P, NT, C], BF16, kind="ExternalInput")
    sel_d = nc.dram_tensor("sel", [P, NT, G], F32, kind="ExternalInput")
    selb_d = nc.dram_tensor("selb", [P, C], F32, kind="ExternalInput")
    aff_d = nc.dram_tensor("aff", [P, 3 * NT], F32, kind="ExternalInput")
    out_d = nc.dram_tensor("out", [BPC, C, N], F32, kind="ExternalOutput")

    with tile.TileContext(nc) as tc:
        with (
            tc.tile_pool(name="consts", bufs=1) as consts,
            tc.tile_pool(name="xp", bufs=2) as xp,
            tc.tile_pool(name="xpbp", bufs=3) as xpbp,
            tc.tile_pool(name="gn", bufs=2) as gn,
            tc.tile_pool(name="hbp", bufs=2) as hbp,
            tc.tile_pool(name="h8p", bufs=2) as h8p,
            tc.tile_pool(name="t8p", bufs=2) as t8p,
            tc.tile_pool(name="ptp", bufs=2) as ptp,
            tc.tile_pool(name="vtp", bufs=2) as vtp,
            tc.tile_pool(name="vep", bufs=2) as vep,
            tc.tile_pool(name="recp", bufs=2) as recp,
            tc.tile_pool(name="tmpp", bufs=2) as tmpp,
            tc.tile_pool(name="resp", bufs=2) as resp,
            tc.tile_pool(name="pa", bufs=2, space="PSUM") as pa,
            tc.tile_pool(name="pcp", bufs=1, space="PSUM") as pcp,
            tc.tile_pool(name="pdp", bufs=1, space="PSUM") as pdp,
        ):
            mt8 = consts.tile([P, NT, C], FP8)
            wvT = consts.tile([P, NT, C], BF16)
            sel = consts.tile([P, NT, G], F32)
            selb = consts.tile([P, C], F32)
            aff = consts.tile([P, 3 * NT], F32)
            ones8 = consts.tile([P, NT, P], FP8)
            ebias = consts.tile([P, 1], F32)

            def emit_weight_loads():
                # SWDGE path runs parallel to the HWDGE x-load at startup
                nc.gpsimd.dma_start(mt8[:], mt8_d.ap())
                nc.gpsimd.dma_start(wvT[:], wvT_d.ap())
                nc.gpsimd.dma_start(sel[:], sel_d.ap())
                nc.gpsimd.dma_start(selb[:], selb_d.ap())
                nc.gpsimd.memset(ones8[:], WSCALE)
                nc.gpsimd.memset(ebias[:], EBIAS)

            def emit_x(img):
                x_sb = xp.tile([P, NT, N], F32, tag="x")
                xr = x_d.ap()[img].rearrange("(t p) n -> p t n", p=P)
                for t in range(NT):
                    for s in range(2):
                        nc.sync.dma_start(
                            x_sb[:, t, s * 512:(s + 1) * 512],
                            xr[:, t, s * 512:(s + 1) * 512])
                return x_sb

            def emit_xpb(x_sb):
                """residual base xpb = x + proj_b on the idle GPSIMD"""
                xpb_sb = xpbp.tile([P, NT, N], F32, tag="xpb")
                for t in range(NT):
                    nc.gpsimd.tensor_scalar_add(
                        xpb_sb[:, t, :], x_sb[:, t, :],
                        aff[:, 2 * NT + t:2 * NT + t + 1])
                return xpb_sb

            def emit_gn_h(x_sb, first=False):
                """GroupNorm stats -> per-channel affine -> hb (bf16), h8."""
                bst = gn.tile([P, NT, 2, 6], F32, tag="bst")
                for t in range(NT):
                    for s in range(2):
                        nc.vector.bn_stats(
                            bst[:, t, s, :], x_sb[:, t, s * 512:(s + 1) * 512])
                cmv = gn.tile([P, NT, 2], F32, tag="cmv")
                for t in range(NT):
                    nc.vector.bn_aggr(cmv[:, t, :], bst[:, t, :, :])
                # ex2 columns: [mean_c, E[x^2]_c]
                ex2 = gn.tile([P, NT, 2], F32, tag="ex2")
                for t in range(NT):
                    nc.vector.tensor_mul(
                        ex2[:, t, 1:2], cmv[:, t, 0:1], cmv[:, t, 0:1])
                    nc.vector.tensor_add(
                        ex2[:, t, 1:2], ex2[:, t, 1:2], cmv[:, t, 1:2])
                    (nc.vector if first else nc.gpsimd).tensor_copy(
                        ex2[:, t, 0:1], cmv[:, t, 0:1])
                # group stats = (1/GS) * sel.T @ ex2 -> psum [G, 2]
                # (pc pool: its ch0 consumer starts late in the image period,
                # so sharing with den would stall den's early accumulation)
                psg = pcp.tile([P, 2], F32, tag="c")
                for t in range(NT):
                    nc.tensor.matmul(psg[0:G, :], sel[:, t, :], ex2[:, t, :],
                                     start=(t == 0), stop=(t == NT - 1))
                # gsb cols: [mean_g, rstd_g, v, tmp]; rows 8..127 zero (pad
                # for matmul).  rstd via DVE-only Newton rsqrt so Exp stays
                # the single ACT table set.
                gsb = gn.tile([P, 4], F32, tag="gsb")
                nc.vector.memset(gsb[:], 0.0)
                nc.vector.tensor_copy(gsb[0:G, 0:1], psg[0:G, 0:1])
                nc.vector.tensor_mul(
                    gsb[0:G, 3:4], gsb[0:G, 0:1], gsb[0:G, 0:1])
                nc.vector.tensor_tensor(
                    gsb[0:G, 2:3], psg[0:G, 1:2], gsb[0:G, 3:4], ALU.subtract)
                nc.vector.tensor_scalar_add(gsb[0:G, 2:3], gsb[0:G, 2:3], EPS)
                nc.vector.reciprocal(gsb[0:G, 3:4], gsb[0:G, 2:3])
                nc.vector.tensor_scalar(
                    gsb[0:G, 1:2], gsb[0:G, 3:4], 1.0, 0.5, ALU.add, ALU.mult)
                for _ in range(2):
                    nc.vector.tensor_mul(
                        gsb[0:G, 3:4], gsb[0:G, 1:2], gsb[0:G, 1:2])
                    nc.vector.tensor_mul(
                        gsb[0:G, 3:4], gsb[0:G, 3:4], gsb[0:G, 2:3])
                    nc.vector.tensor_scalar(
                        gsb[0:G, 3:4], gsb[0:G, 3:4], -0.5, 1.5,
                        ALU.mult, ALU.add)
                    nc.vector.tensor_mul(
                        gsb[0:G, 1:2], gsb[0:G, 1:2], gsb[0:G, 3:4])
                # broadcast group -> channel: selb.T @ gsb -> [c, (mean,rstd)]
                AB = gn.tile([P, NT, 2], F32, tag="AB")
                hb = hbp.tile([P, NT, N], BF16, tag="hb")
                for cu in range(NT):
                    psc = pcp.tile([P, 2], F32, tag="c")
                    nc.tensor.matmul(psc[:], selb[:, cu * P:(cu + 1) * P],
                                     gsb[:, 0:2], start=True, stop=True)
                    # A = rstd*w ; B = b - mean*A
                    nc.vector.tensor_mul(
                        AB[:, cu, 0:1], psc[:, 1:2], aff[:, cu:cu + 1])
                    nc.vector.tensor_mul(
                        AB[:, cu, 1:2], psc[:, 0:1], AB[:, cu, 0:1])
                    nc.vector.tensor_tensor(
                        AB[:, cu, 1:2], aff[:, NT + cu:NT + cu + 1],
                        AB[:, cu, 1:2], ALU.subtract)
                    # hb = A*x + B   (bf16)
                    nc.vector.tensor_scalar(
                        hb[:, cu, :], x_sb[:, cu, :],
                        AB[:, cu, 0:1], AB[:, cu, 1:2], ALU.mult, ALU.add)
                # h8: fp8 shadow of h for the t/logits DoubleRow path.
                # image 0: Pool is on the critical path to the first logits,
                # so build h8 on DVE there
                h8 = h8p.tile([P, NT, N], FP8, tag="h8")
                if first:
                    nc.vector.tensor_copy(h8[:], hb[:])
                else:
                    nc.gpsimd.tensor_copy(h8[:], hb[:])
                return hb, h8

            def make_qkv(hb, h8):
                """Per-image QKV emitters: t = M h (fp8 DR) and the fp8
                split vto8/vtoe8 of (WpWv) h.  Returned as closures so they
                can be fired inside the previous image's attention stream."""
                t8 = t8p.tile([P, NT, N], FP8, tag="t8")
                vto8 = vtp.tile([P, G, C], FP8, tag="vto8")
                vtoe8 = vep.tile([P, G, C], FP8, tag="vtoe8")

                def t_group(chalf):
                    pst = pa.tile([P, N], F32, tag="a")
                    for mh in range(2):
                        nc.tensor.matmul(
                            pst[:, mh * 512:(mh + 1) * 512],
                            mt8[:, 0:NT, chalf * P:(chalf + 1) * P],
                            h8[:, 0:NT, mh * 512:(mh + 1) * 512],
                            start=True, stop=True, perf_mode=DR)
                    nc.scalar.copy(t8[:, chalf, :], pst[:])

                def vt_group(j):
                    # 4 m-blocks per [P, 1024] psum tile: one wide fp8 copy
                    # + one wide residual STT.  pd pool: den's bank is free
                    # right after recip, exactly when the vt matmuls fire --
                    # and keeping vt out of pa unblocks the next psl tiles
                    # (and so the next exp stream) much earlier.
                    psv = pdp.tile([P, N], F32, tag="d")
                    pv = psv[:].rearrange("p (q c) -> p q c", q=4)
                    for half in range(4):
                        mb = 4 * j + half
                        for kt in range(NT):
                            nc.tensor.matmul(
                                pv[:, half, :],
                                hb[:, kt, mb * P:(mb + 1) * P],
                                wvT[:, kt, :],
                                start=(kt == 0), stop=(kt == NT - 1))
                    nc.scalar.copy(vto8[:, 4 * j:4 * j + 4, :], pv[:])
                    # fp8 residual: vtoe8 = (psv + 0) - vto8
                    nc.vector.scalar_tensor_tensor(
                        vtoe8[:, 4 * j:4 * j + 4, :], pv[:], 0.0,
                        vto8[:, 4 * j:4 * j + 4, :], ALU.add, ALU.subtract)

                return t8, vto8, vtoe8, t_group, vt_group

            def emit_attn(img, h8, t8, vto8, vtoe8, xpb_sb, nxt=None):
                """Fused logits + attention-output stage.

                The den (all-ones DR) and outT-chalf0 accumulations lag one
                exp-pair behind the logits/exp stream, so by the time the
                last exp retires only one j-term plus the normalize tail
                remains.  l-matmuls are kept two mk ahead so the ACT exp
                stream never stalls on PE.  chalf1 re-reads pt8 from SBUF
                after the stream; the next image's t/vt matmuls are
                interleaved to keep ACT fed between exp streams."""
                pt8 = ptp.tile([P, G, N], FP8, tag="pt")
                pc0 = pcp.tile([P, N], F32, tag="c")
                pd = pdp.tile([P, N], F32, tag="d")
                psl_live = {}

                def lmm(mk):
                    psl = pa.tile([P, N], F32, tag="a")
                    for nh in range(2):
                        nc.tensor.matmul(
                            psl[:, nh * 512:(nh + 1) * 512],
                            t8[:, 0:NT, mk * P:(mk + 1) * P],
                            h8[:, 0:NT, nh * 512:(nh + 1) * 512],
                            start=True, stop=True, perf_mode=DR)
                    psl_live[mk] = psl

                def expo(mk):
                    nc.scalar.activation(pt8[:, mk, :], psl_live.pop(mk)[:],
                                         AF.Exp, bias=ebias[:], scale=SCALE)

                def av_terms(po, jp, chalf, start, stop):
                    for nh in range(2):
                        for src in (vto8, vtoe8):
                            nc.tensor.matmul(
                                po[:, nh * 512:(nh + 1) * 512],
                                src[:, 2 * jp:2 * jp + 2,
                                    chalf * P:(chalf + 1) * P],
                                pt8[:, 2 * jp:2 * jp + 2,
                                    nh * 512:(nh + 1) * 512],
                                start=(start and src is vto8),
                                stop=(stop and src is vtoe8),
                                perf_mode=DR)

                lmm(0), expo(0), lmm(1), expo(1)
                for jp in range(4):
                    if jp < 3:
                        lmm(2 * jp + 2), expo(2 * jp + 2)
                        lmm(2 * jp + 3), expo(2 * jp + 3)
                    for nh in range(2):
                        nc.tensor.matmul(
                            pd[:, nh * 512:(nh + 1) * 512],
                            ones8[:, 0:NT, :],
                            pt8[:, 2 * jp:2 * jp + 2,
                                nh * 512:(nh + 1) * 512],
                            start=(jp == 0), stop=(jp == 3), perf_mode=DR)
                    av_terms(pc0, jp, 0, start=(jp == 0), stop=(jp == 3))

                rec = recp.tile([P, N], F32, tag="rec")
                tmp = tmpp.tile([P, NT, N], F32, tag="tmp")
                res = resp.tile([P, NT, N], F32, tag="res")
                outr = out_d.ap()[img].rearrange("(t p) n -> p t n", p=P)

                nc.vector.reciprocal(rec[:], pd[:])
                nc.vector.tensor_tensor(tmp[:, 0, :], pc0[:], rec[:],
                                        ALU.mult)
                nc.gpsimd.tensor_tensor(res[:, 0, :], tmp[:, 0, :],
                                        xpb_sb[:, 0, :], ALU.add)
                # next image's t-matmuls first: they unblock the ACT t-copies
                # right after the exp stream drains
                if nxt is not None:
                    nxt[3](0)
                    nxt[3](1)
                for s in range(2):
                    nc.sync.dma_start(outr[:, 0, s * 512:(s + 1) * 512],
                                      res[:, 0, s * 512:(s + 1) * 512])
                pc1 = pcp.tile([P, N], F32, tag="c")
                for jp in range(4):
                    av_terms(pc1, jp, 1, start=(jp == 0), stop=(jp == 3))
                if nxt is not None:
                    nxt[4](0)
                nc.vector.tensor_tensor(tmp[:, 1, :], pc1[:], rec[:],
                                        ALU.mult)
                nc.gpsimd.tensor_tensor(res[:, 1, :], tmp[:, 1, :],
                                        xpb_sb[:, 1, :], ALU.add)
                if nxt is not None:
                    nxt[4](1)
                for s in range(2):
                    nc.sync.dma_start(outr[:, 1, s * 512:(s + 1) * 512],
                                      res[:, 1, s * 512:(s + 1) * 512])

            # software-pipelined emission with two-image lookahead: image
            # i+1's t/vt matmuls fire inside attn(i); gn(i+2) is emitted
            # after attn(i) so its products are long ready by attn(i+1).
            imgs = [i % BPC for i in range(BPC * repeat)]
            # aff is consumed late, so it rides the SWDGE queue, keeping
            # HWDGE x-only at startup
            nc.gpsimd.dma_start(aff[:], aff_d.ap())
            emit_weight_loads()
            x0 = emit_x(imgs[0])
            # warmup: trigger the Exp ACT-table DMA (~1.3us) right after the
            # x-load dispatches so it is resident long before image 0's
            # first softmax exp
            warm = consts.tile([P, 1], F32)
            nc.vector.memset(warm[:], 0.0)
            nc.scalar.activation(warm[:], warm[:], AF.Exp)
            hb0, h80 = emit_gn_h(x0, first=True)
            xpb0 = emit_xpb(x0)
            q0 = make_qkv(hb0, h80)
            q0[3](0), q0[3](1), q0[4](0), q0[4](1)
            cur = (imgs[0], xpb0, h80, q0)
            nxt_gn = None
            if len(imgs) > 1:
                x1 = emit_x(imgs[1])
                hb1, h81 = emit_gn_h(x1)
                xpb1 = emit_xpb(x1)
                nxt_gn = (imgs[1], xpb1, h81, hb1)
            for idx in range(len(imgs)):
                img, xpb_sb, h8, (t8, vto8, vtoe8, _tg, _vg) = cur
                nxt = None
                xn = None
                if nxt_gn is not None and idx + 2 < len(imgs):
                    # prefetch x(i+2) before attn(i): the DMA must not gate
                    # the GroupNorm stats chain emitted after attn(i)
                    xn = emit_x(imgs[idx + 2])
                if nxt_gn is not None:
                    n_img, n_xpb, n_h8, n_hb = nxt_gn
                    nxt = make_qkv(n_hb, n_h8)
                    cur = (n_img, n_xpb, n_h8, nxt)
                emit_attn(img, h8, t8, vto8, vtoe8, xpb_sb, nxt=nxt)
                if xn is not None:
                    hbn, h8n = emit_gn_h(xn)
                    xpbn = emit_xpb(xn)
                    nxt_gn = (imgs[idx + 2], xpbn, h8n, hbn)
                else:
                    nxt_gn = None

    nc.compile()
    return nc


def _build_runner(repeat=1):
    """Build nc once and wrap it in a persistent jitted 8-core SPMD callable."""
    nc = _build_program(repeat)
    bass2jax.install_neuronx_cc_hook()

    partition_name = (nc.partition_id_tensor.name
                      if nc.partition_id_tensor else None)
    in_names, out_names, out_avals = [], [], []
    for alloc in nc.m.functions[0].allocations:
        if not isinstance(alloc, mybir.MemoryLocationSet):
            continue
        name = alloc.memorylocations[0].name
        if alloc.kind == "ExternalInput":
            if name != partition_name:
                in_names.append(name)
        elif alloc.kind == "ExternalOutput":
            out_names.append(name)
            out_avals.append(jax.core.ShapedArray(
                tuple(alloc.tensor_shape), mybir.dt.np(alloc.dtype)))
    n_params = len(in_names)
    all_in_names = tuple(in_names) + tuple(out_names)
    if partition_name is not None:
        all_in_names = all_in_names + (partition_name,)

    def _body(*args):
        operands = list(args)
        if partition_name is not None:
            operands.append(bass2jax.partition_id_tensor())
        return tuple(bass2jax._bass_exec_p.bind(
            *operands,
            out_avals=tuple(out_avals),
            in_names=all_in_names,
            out_names=tuple(out_names),
            lowering_input_output_aliases=(),
            sim_require_finite=True,
            sim_require_nnan=True,
            nc=nc,
        ))

    devices = jax.devices()[:NCORES]
    mesh = Mesh(np.asarray(devices), ("core",))
    nin = n_params + len(out_names)
    sharded = jax.jit(
        shard_map(_body, mesh=mesh,
                  in_specs=(PartitionSpec("core"),) * nin,
                  out_specs=(PartitionSpec("core"),) * len(out_names),
                  check_rep=False),
        keep_unused=True,
    )
    from jax.sharding import NamedSharding
    shard = NamedSharding(mesh, PartitionSpec("core"))
    zeros_dev = [
        jax.device_put(
            np.zeros((NCORES * a.shape[0], *a.shape[1:]), a.dtype), shard)
        for a in out_avals
    ]
    return {"sharded": sharded, "in_names": in_names,
            "out_names": out_names, "out_avals": out_avals,
            "zeros_dev": zeros_dev, "mesh": mesh, "nc": nc}


def _get_runner(repeat=1):
    global _cached
    if _cached is None:
        _cached = {}
    if repeat not in _cached:
        _cached[repeat] = _build_runner(repeat)
    return _cached[repeat]


def _run(in_maps):
    r = _get_runner()
    sharded, in_names, out_names, out_avals, zeros_dev = (
        r["sharded"], r["in_names"], r["out_names"], r["out_avals"],
        r["zeros_dev"])
    concat_in = [
        np.concatenate([np.asarray(m[name]) for m in in_maps], axis=0)
        for name in in_names
    ]
    out_arrs = sharded(*concat_in, *zeros_dev)
    return {
        name: np.asarray(out_arrs[i]).reshape(
            NCORES, *out_avals[i].shape)
        for i, name in enumerate(out_names)
    }


def _pack_c(v):
    # [C] -> [P, NT] with c = t*128 + p
    return np.ascontiguousarray(v.reshape(NT, P).T)


def _pack_w(wT, dtype):
    # [C, O] -> [P, NT, O] with c = t*128 + p
    o = wT.shape[1]
    return np.ascontiguousarray(
        wT.reshape(NT, P, o).transpose(1, 0, 2)).astype(dtype)


def make_in_maps(x, norm_w, norm_b, qkv_w, proj_w, proj_b):
    x = np.asarray(x, dtype=np.float32)
    norm_w = np.asarray(norm_w, dtype=np.float32)
    norm_b = np.asarray(norm_b, dtype=np.float32)
    qkv_w = np.asarray(qkv_w, dtype=np.float64)
    proj_w = np.asarray(proj_w, dtype=np.float64)
    proj_b = np.asarray(proj_b, dtype=np.float32)

    # M = Wq^T Wk; lhsT for t = M @ h is M^T in [c_in, c_out] layout
    M = qkv_w[:C].T @ qkv_w[C:2 * C]
    mt8 = _pack_w((WSCALE * M.T).astype(np.float32),
                  ml_dtypes.float8_e4m3fn)       # [P, NT, C]
    # fold proj into V: (Wp @ Wv), prescaled to match the ones column
    wvp = (proj_w @ qkv_w[2 * C:])
    wvT = _pack_w((WSCALE * wvp.T).astype(np.float32),
                  ml_dtypes.bfloat16)            # [P, NT, C]

    cidx = np.arange(C)
    sel = np.zeros((P, NT, G), np.float32)
    sel[cidx % P, cidx // P, cidx // GS] = 1.0 / GS
    selb = np.zeros((P, C), np.float32)
    selb[cidx // GS, cidx] = 1.0

    aff = np.concatenate(
        [_pack_c(norm_w), _pack_c(norm_b), _pack_c(proj_b)],
        axis=1).astype(np.float32)               # [P, 6]

    xr = x.reshape(B, C, N)
    shared = {"mt8": mt8, "wvT": wvT, "sel": sel, "selb": selb, "aff": aff}
    return [
        {"x": np.ascontiguousarray(xr[c * BPC:(c + 1) * BPC]), **shared}
        for c in range(NCORES)
    ]


def kernel(x, norm_w, norm_b, qkv_w, proj_w, proj_b):
    in_maps = make_in_maps(x, norm_w, norm_b, qkv_w, proj_w, proj_b)
    outs = _run(in_maps)
    return outs["out"].reshape(B, C, 32, 32)
